# revision 24
# baseline (speedup 1.0000x reference)
"""Trainium2 Bass kernel for nn_AttentionCropLayer (attention crop + bilinear
resize), data-parallel over 8 NeuronCores.

Reformulation (validated vs the jax reference):
  For each sample, the soft-masked crop + align-corners bilinear resize is
  exactly  out[c] = Rt.T @ X[c] @ Ct  with
    Rt[i,j] = mrow[i] * hat(i - sr[j]),  Ct[k,m] = mcol[k] * hat(k - sc[m]),
    hat(d) = relu(1 - |d|),
    sr[j] = w_off + j*(w_end-w_off-1)/107  (and likewise sc),
  because the reference's integer crop box satisfies w_off >= 26 > 0 so the
  r0/r1 gather taps are exactly the two nonzeros of the hat function, and the
  sigmoid box masks fold into the interpolation matrices diagonally.

v2 layout/engine plan (vs v1: 371us HW):
  - DRAM in/out are BF16 (host casts); halves DMA bytes, kills the Pool
    f32->bf16 casts.  Tolerance budget: 2e-2; measured ~3e-3 in v1.
  - hat build per slab: d = iota - (srT+1) on Pool (f32 in, bf16 out),
    a = |d|-1 via one fused abs_max+add op on DVE (all-bf16, 2x mode).
  - per sample: PE transposes a slices; rt/ct finalize on DVE as
    tensor_scalar(psum, -mask[P,1], 0.0, mult, max) (mask+relu fused, the
    per-partition scalar is exempt from 2x dtype rules).
  - t1 evac alternates DVE/ACT; output evac on ACT.
  - PE stream software-pipelined: t_r(k), t_c(k), mm1(k), mm2(k-1) so PE
    never stalls on the psum-evac round trips.
  - input DMA on sync(SP) ring, output DMA on scalar(ACT) ring.
"""
import numpy as np
import ml_dtypes

import concourse.bass as bass
import concourse.tile as tile
from concourse import mybir
from concourse.alu_op_type import AluOpType as Op

F32 = mybir.dt.float32
BF16 = mybir.dt.bfloat16
I32 = mybir.dt.int32
AF = mybir.ActivationFunctionType
P = 108
N_CORES = 8
S = 128   # samples per core
SL = 16   # slab size

_ctr = [0]


def _split_multi_waits(nc):
    """This container's walrus accepts at most ONE sync-wait per instruction
    (none on Drain). Move excess waits onto preceding same-engine no-ops."""
    moved = 0
    for func in nc.m.functions:
        for blk in func.blocks:
            out_insts = []
            changed = False
            for inst in blk.instructions:
                si = inst.sync_info
                waits = list(si.on_wait) if (si and si.on_wait) else []
                limit = 0 if inst.opcode == "Drain" else 1
                if len(waits) > limit:
                    # keep the most-recently-added wait (usually the
                    # latest-firing producer) on the real instruction; earlier
                    # waits ride on NoOps where their SEQ-blocking is cheap
                    # because they usually fire first.
                    keep = waits[len(waits) - limit:]
                    excess = waits[:len(waits) - limit]
                    for w in excess:
                        _ctr[0] += 1
                        nop = mybir.InstNoOp(
                            name=f"waitsplit-{_ctr[0]}",
                            sync_info=mybir.SyncInfo(on_wait=[w], on_update=[]),
                            bass_nofuse=True,
                            engine=inst.engine,
                        )
                        out_insts.append(nop)
                        moved += 1
                    upd = list(si.on_update) if si.on_update else []
                    inst.sync_info = mybir.SyncInfo(on_wait=keep, on_update=upd)
                    changed = True
                out_insts.append(inst)
            if changed:
                try:
                    blk.instructions = out_insts
                except Exception:
                    blk.clear_instructions()
                    for i in out_insts:
                        blk.add_instruction(i)
    return moved


def _build():
    nslabs = S // SL
    nc = bass.Bass()
    images = nc.declare_dram_parameter("images", [S, 3, P, P], BF16, isOutput=False)
    locs = nc.declare_dram_parameter("locs", [S, 3], F32, isOutput=False)
    iota_d = nc.declare_dram_parameter("iota", [128, P], F32, isOutput=False)
    idf_d = nc.declare_dram_parameter("idf", [128, 128], F32, isOutput=False)
    idb_d = nc.declare_dram_parameter("idb", [128, 128], BF16, isOutput=False)
    out = nc.declare_dram_parameter("out", [S, 3, P, P], BF16, isOutput=True)

    with tile.TileContext(nc) as tc:
        with (
            tc.tile_pool(name="consts", bufs=1) as consts,
            tc.tile_pool(name="setup", bufs=1) as setup,
            tc.tile_pool(name="xpool", bufs=3) as xpool,
            tc.tile_pool(name="hat", bufs=3) as hatp,
            tc.tile_pool(name="ostage", bufs=2) as ostage_p,
            tc.tile_pool(name="samp", bufs=3) as samp,
        ):
            iota = consts.tile([128, P], F32)
            nc.sync.dma_start(out=iota, in_=iota_d[:, :])
            idf = consts.tile([128, 128], F32)
            nc.sync.dma_start(out=idf, in_=idf_d[:, :])
            idb = consts.tile([128, 128], BF16)
            nc.sync.dma_start(out=idb, in_=idb_d[:, :])

            lt = setup.tile([S, 3], F32)
            nc.sync.dma_start(out=lt, in_=locs[:, :])

            def col(t, j):
                return t[:, j:j + 1]

            # trunc(m*l + 0.5) == RNE-convert(m*l)
            tx = setup.tile([S, 1], F32)
            ty = setup.tile([S, 1], F32)
            tlh = setup.tile([S, 1], F32)
            for j, m, t in ((0, 27.0, tx), (1, 27.0, ty), (2, 7.0, tlh)):
                v = setup.tile([S, 1], F32, tag="v_scaled")
                nc.vector.tensor_scalar(v, col(lt, j), m, None, Op.mult)
                vi = setup.tile([S, 1], I32, tag="v_int")
                nc.vector.tensor_copy(vi, v)
                nc.vector.tensor_copy(t, vi)

            # w_off = tx - tlh + 33 ; w_end = min(tx + tlh + 75, 108)
            w_off = setup.tile([S, 1], F32)
            nc.vector.scalar_tensor_tensor(w_off, tx, 33.0, tlh, Op.add, Op.subtract)
            w_end = setup.tile([S, 1], F32)
            nc.vector.scalar_tensor_tensor(w_end, tx, 75.0, tlh, Op.add, Op.add)
            nc.vector.tensor_scalar(w_end, w_end, 108.0, None, Op.min)
            h_off = setup.tile([S, 1], F32)
            nc.vector.scalar_tensor_tensor(h_off, ty, 33.0, tlh, Op.add, Op.subtract)
            h_end = setup.tile([S, 1], F32)
            nc.vector.scalar_tensor_tensor(h_end, ty, 75.0, tlh, Op.add, Op.add)
            nc.vector.tensor_scalar(h_end, h_end, 108.0, None, Op.min)

            # sr = iota * (w_end-w_off-1)/107 + w_off
            sr = setup.tile([S, P], F32)
            sc = setup.tile([S, P], F32)
            for off, end, dst in ((w_off, w_end, sr), (h_off, h_end, sc)):
                a = setup.tile([S, 1], F32, tag="a_slope")
                nc.vector.scalar_tensor_tensor(a, end, -1.0, off, Op.add, Op.subtract)
                nc.vector.tensor_scalar(a, a, 1.0 / 107.0, None, Op.mult)
                nc.vector.tensor_scalar(dst, iota[:S, :], a, off, Op.mult, Op.add)

            # negated masks: m_neg = sig(10(i-end)) - sig(10(i-off))
            mrow_n = setup.tile([S, P], F32)
            mcol_n = setup.tile([S, P], F32)
            for off, end, dst in ((w_off, w_end, mrow_n), (h_off, h_end, mcol_n)):
                b_off = setup.tile([S, 1], F32, tag="b_off")
                nc.vector.tensor_scalar(b_off, off, -10.0, None, Op.mult)
                b_end = setup.tile([S, 1], F32, tag="b_end")
                nc.vector.tensor_scalar(b_end, end, -10.0, None, Op.mult)
                s_off = setup.tile([S, P], F32, tag="s_off")
                nc.scalar.activation(s_off, iota[:S, :], AF.Sigmoid, bias=b_off, scale=10.0)
                s_end = setup.tile([S, P], F32, tag="s_end")
                nc.scalar.activation(s_end, iota[:S, :], AF.Sigmoid, bias=b_end, scale=10.0)
                nc.vector.tensor_sub(dst, s_end, s_off)

            # transposed per-sample params
            srT = setup.tile([P, S], F32)
            scT = setup.tile([P, S], F32)
            mrowT_n = setup.tile([P, S], F32)
            mcolT_n = setup.tile([P, S], F32)
            with tc.tile_pool(name="setup_ps", bufs=2, space="PSUM") as setup_ps:
                for src_t, dst in (
                    (sr, srT), (sc, scT), (mrow_n, mrowT_n), (mcol_n, mcolT_n),
                ):
                    pst = setup_ps.tile([P, S], F32, tag="setup_tr")
                    nc.tensor.transpose(pst, src_t, idf[:S, :S])
                    nc.vector.tensor_copy(dst, pst)

            import contextlib
            _ps_stack = contextlib.ExitStack()
            ps_tr = _ps_stack.enter_context(
                tc.tile_pool(name="ps_tr", bufs=2, space="PSUM"))
            ps_mm = _ps_stack.enter_context(
                tc.tile_pool(name="ps_mm", bufs=3, space="PSUM"))

            iota_b = iota[:P, :].unsqueeze(1).broadcast_to([P, SL, P])

            NCH = 4          # hat-build chunks per slab
            CW = SL // NCH   # chunk width (samples)

            def emit_slab_dma(t):
                """DMA-in + Pool d-builds for slab t; returns tiles."""
                s0 = t * SL
                x_b = xpool.tile([P, SL, 3, P], BF16, tag="x_b")
                nc.sync.dma_start(out=x_b,
                                  in_=images[s0:s0 + SL, :, :, :].transpose([2, 0, 1, 3]))
                # d = i - sr  (f32 ins, bf16 out), on Pool
                d_r = hatp.tile([P, SL, P], BF16, tag="d_r")
                d_c = hatp.tile([P, SL, P], BF16, tag="d_c")
                srT_b = srT[:, s0:s0 + SL].unsqueeze(2).broadcast_to([P, SL, P])
                scT_b = scT[:, s0:s0 + SL].unsqueeze(2).broadcast_to([P, SL, P])
                nc.gpsimd.tensor_sub(d_r, iota_b, srT_b)
                nc.gpsimd.tensor_sub(d_c, iota_b, scT_b)
                a_r = hatp.tile([P, SL, P], BF16, tag="a_r")
                a_c = hatp.tile([P, SL, P], BF16, tag="a_c")
                e_r = hatp.tile([P, SL, P], BF16, tag="e_r")
                e_c = hatp.tile([P, SL, P], BF16, tag="e_c")
                return x_b, d_r, d_c, e_r, e_c, a_r, a_c

            def emit_hat_chunk(tiles, q):
                """a = |d| - 1 = max(d-1, -d-1) for chunk q of a slab; on Pool
                (chunked) so they never delay the DVE finalize chain."""
                _, d_r, d_c, e_r, e_c, a_r, a_c = tiles
                sls = slice(q * CW, (q + 1) * CW)
                nc.vector.tensor_scalar(e_r[:, sls], d_r[:, sls], -1.0, -1.0,
                                        Op.mult, Op.add)
                nc.vector.scalar_tensor_tensor(a_r[:, sls], d_r[:, sls], -1.0,
                                               e_r[:, sls], Op.add, Op.max)
                nc.vector.tensor_scalar(e_c[:, sls], d_c[:, sls], -1.0, -1.0,
                                        Op.mult, Op.add)
                nc.vector.scalar_tensor_tensor(a_c[:, sls], d_c[:, sls], -1.0,
                                               e_c[:, sls], Op.add, Op.max)

            slabs = {}
            slabs[0] = emit_slab_dma(0)
            for q in range(NCH):
                emit_hat_chunk(slabs[0], q)
            if nslabs > 1:
                slabs[1] = emit_slab_dma(1)
            ostages = {}

            def emit_tr_fins(k):
                """PE transposes + DVE mask-relu finalizes for sample k."""
                t, sl = divmod(k, SL)
                a_r, a_c = slabs[t][5], slabs[t][6]
                tr_ps = ps_tr.tile([P, 2, P], BF16, tag="tr")
                nc.tensor.transpose(tr_ps[:, 0], a_r[:, sl], idb[:P, :P])
                nc.tensor.transpose(tr_ps[:, 1], a_c[:, sl], idb[:P, :P])
                rt = samp.tile([P, P], BF16, tag="rt")
                nc.vector.tensor_scalar(rt, tr_ps[:, 0], mrowT_n[:, k:k + 1],
                                        0.0, Op.mult, Op.max)
                ct = samp.tile([P, P], BF16, tag="ct")
                nc.vector.tensor_scalar(ct, tr_ps[:, 1], mcolT_n[:, k:k + 1],
                                        0.0, Op.mult, Op.max)
                return rt, ct

            pend = emit_tr_fins(0)
            prev = None  # (t1, ct, o_stage_tile, sl_prev, t_prev)

            for k in range(S):
                t, sl = divmod(k, SL)
                if sl == 0:
                    if t + 2 < nslabs:
                        slabs[t + 2] = emit_slab_dma(t + 2)
                    ostages[t] = ostage_p.tile([P, SL, 3, P], BF16,
                                               name="o_stage", tag="o_stage")
                if t + 1 < nslabs and sl % CW == 1:
                    emit_hat_chunk(slabs[t + 1], sl // CW)

                # lookahead: transposes + finalizes for sample k+1, so the
                # DVE round trip hides behind this sample's matmuls
                nxt = emit_tr_fins(k + 1) if k + 1 < S else None

                # mm1: t1 = X^T Rt per channel
                x_b = slabs[t][0]
                rt, ct = pend
                t1_ps = ps_mm.tile([P, 3, P], F32, tag="t1")
                for c in range(3):
                    nc.tensor.matmul(t1_ps[:, c], x_b[:, sl, c], rt,
                                     start=True, stop=True)
                t1 = samp.tile([P, 3, P], BF16, tag="t1sb")
                nc.scalar.activation(t1, t1_ps, AF.Copy)

                # mm2 for previous sample (software pipeline, lag 1)
                if prev is not None:
                    pt1, pct, postage, psl, ptt = prev
                    o_ps = ps_mm.tile([P, 3, P], F32, tag="o")
                    for c in range(3):
                        nc.tensor.matmul(o_ps[:, c], pt1[:, c], pct,
                                         start=True, stop=True)
                    if k % 2 == 0:
                        nc.vector.tensor_copy(postage[:, psl], o_ps)
                    else:
                        nc.scalar.activation(postage[:, psl], o_ps, AF.Copy)
                    if psl == SL - 1:
                        p0 = ptt * SL
                        nc.scalar.dma_start(
                            out=out[p0:p0 + SL, :, :, :].transpose([2, 0, 1, 3]),
                            in_=postage)
                prev = (t1, ct, ostages[t], sl, t)
                pend = nxt

            # epilogue: flush last sample
            pt1, pct, postage, psl, ptt = prev
            o_ps = ps_mm.tile([P, 3, P], F32, tag="o")
            for c in range(3):
                nc.tensor.matmul(o_ps[:, c], pt1[:, c], pct, start=True, stop=True)
            nc.scalar.activation(postage[:, psl], o_ps, AF.Copy)
            p0 = ptt * SL
            nc.scalar.dma_start(
                out=out[p0:p0 + SL, :, :, :].transpose([2, 0, 1, 3]),
                in_=postage)
            _ps_stack.close()
    return nc


def _host_constants():
    iota = np.tile(np.arange(P, dtype=np.float32), (128, 1))
    idf = np.eye(128, dtype=np.float32)
    idb = np.eye(128, dtype=ml_dtypes.bfloat16)
    return {"iota": iota, "idf": idf, "idb": idb}


_cached_nc = None


def _get_nc():
    global _cached_nc
    if _cached_nc is None:
        nc = _build()
        _split_multi_waits(nc)
        _cached_nc = nc
    return _cached_nc


def kernel(images: np.ndarray, locs: np.ndarray) -> np.ndarray:
    from concourse.bass_utils import run_bass_kernel_spmd

    images = np.asarray(images)
    locs = np.ascontiguousarray(np.asarray(locs, dtype=np.float32))
    assert images.shape == (N_CORES * S, 3, P, P), images.shape
    assert locs.shape == (N_CORES * S, 3), locs.shape
    images_b = np.ascontiguousarray(images.astype(ml_dtypes.bfloat16))

    nc = _get_nc()
    consts = _host_constants()
    in_maps = [
        {
            "images": images_b[c * S:(c + 1) * S],
            "locs": locs[c * S:(c + 1) * S],
            **consts,
        }
        for c in range(N_CORES)
    ]
    res = run_bass_kernel_spmd(nc, in_maps, list(range(N_CORES)))
    return np.concatenate(
        [res.results[c]["out"] for c in range(N_CORES)], axis=0
    ).astype(np.float32)
